# revision 30
# baseline (speedup 1.0000x reference)
"""Multi-head causal attention (scores = K @ Q^T variant) on 8 TRN2 NeuronCores.

Head-parallel sharding: core c computes heads (2c, 2c+1) end-to-end and the
host concatenates the per-core [T, 128] outputs along the feature axis.

Per-core kernel layout notes:
  - Host passes x transposed ([D, T]) and pre-cast to bf16 so every
    projection matmul has the contraction dim (d) on SBUF partitions with
    zero on-device transposes.
  - Q^T/K^T are stored [128, T] with head0 on partitions 0-63 and head1 on
    64-127, letting the S^T matmuls for both heads run concurrently on
    disjoint PE row-groups (tile_position).
  - Scores are computed transposed (S^T[j, i] = Q_j . K_i) so that the AV
    contraction (over j) lands on the partition axis with no transposes.
  - V is stored in natural layout with a fused ones-column ([V | 1]) so a
    single AV matmul produces both the weighted sum and the softmax
    denominator (PSUM row 64).
  - Wq is pre-scaled on the host by A' = 128*log2(e)/sqrt(HS) so the scores
    PSUM holds 128*log2(exp(s/sqrt(HS))).  The exp is then split across two
    engines to halve the softmax bottleneck:
      * ACT tiles: activation(Exp, scale=ln2/128) - exact.
      * DVE tiles: one tensor_scalar(+B') writing int16 into a bitcast view
        of the bf16 es tile - the int16 bits ARE the bf16 representation of
        2^(ps/128) up to a piecewise-linear mantissa approximation
        (Schraudolph).  Max exp error ~3.5%, which softmax normalization
        mostly cancels; i-block 0 (rows with few keys, worst averaging) is
        pinned to the exact ACT path.  Measured end-to-end rel err ~6e-3.
  - Diagonal-block causal masking runs on the otherwise-idle GPSIMD engine
    (affine_select with fill=0 directly on the es tile).
  - Matmul operands are bf16; PSUM accumulation and normalization are fp32.
  - Projection chunk k and attention i-block k are emitted interleaved
    (i-block k only needs x columns < 512*(k+1)) from one pool set sized to
    exactly 8 PSUM banks, so projections and attention overlap instead of
    serializing on a pool boundary.
"""

import numpy as np

T, D, H, HS = 4096, 1024, 16, 64
NCORES = 8
HPC = H // NCORES  # heads per core = 2
DC = D // 128      # 8 contraction chunks
TC = T // 512      # 8 t-chunks for projections
IB = T // 512      # 8 i-blocks (512 output rows each)
JBN = T // 128     # 32 j-blocks (128 keys each)

# Schraudolph constants: ps = 128*log2(exp(s/8)); es_bits = round(ps + B')
APRIME = 128.0 * np.log2(np.e) / 8.0
CSHIFT = 0.0352                      # minimax relative-error centering
BPRIME = 128.0 * (127.0 - CSHIFT)
LN2_128 = float(np.log(2.0) / 128.0)

_cached_nc = None


def _emit(tc, nc, xT, w6, out):
    import concourse.bass as bass  # noqa: F401
    import concourse.mybir as mybir

    f32 = mybir.dt.float32
    bf16 = mybir.dt.bfloat16
    i16 = mybir.dt.int16
    Exp = mybir.ActivationFunctionType.Exp
    ne = mybir.AluOpType.not_equal
    ge = mybir.AluOpType.is_ge
    add = mybir.AluOpType.add

    with (
        tc.tile_pool(name="const", bufs=1) as constp,
        tc.tile_pool(name="wpool", bufs=1) as wpool,
        tc.tile_pool(name="bigp", bufs=1) as bigp,
        tc.tile_pool(name="xpool", bufs=4) as xpool,
        tc.tile_pool(name="vtp", bufs=2) as vtp,
        tc.tile_pool(name="esp", bufs=8) as esp,
        tc.tile_pool(name="finp", bufs=4) as finp,
        # PSUM budget (8 banks total): s 2x2 + o 2x1 + p 2x1 (shared with
        # the V-transpose tiles so projections get double-buffering).
        tc.tile_pool(name="sp", bufs=2, space="PSUM") as sp,
        tc.tile_pool(name="op", bufs=2, space="PSUM") as op,
        tc.tile_pool(name="pp", bufs=2, space="PSUM") as pp,
    ):
        # ---- input DMAs for weights + first x chunk go first ------------
        w6r = w6.rearrange("(dc p) f -> p dc f", p=128)
        xTr = xT.rearrange("(dc p) t -> p dc t", p=128)
        w6sb = wpool.tile([128, DC, 6 * HS], bf16)
        xts = []
        xt0 = xpool.tile([128, DC, 512], bf16, tag="xt", name="xt0")
        # Per-dc interleave on two DGE queues (x on sync, w6 on scalar) so
        # the dc-k projection matmuls start as soon as pair k has landed,
        # with neither queue blocked behind a bulk transfer.
        for dc in range(DC):
            nc.sync.dma_start(out=xt0[:, dc, :], in_=xTr[:, dc, 0:512])
            nc.scalar.dma_start(out=w6sb[:, dc, :], in_=w6r[:, dc, :])
        xts.append(xt0)

        # ---- constants (gpsimd; overlaps the DMAs) ----------------------
        # id128: full 128x128 identity for the combined V transposes.
        id128 = constp.tile([128, 128], bf16)
        nc.gpsimd.memset(id128, 0.0)
        nc.gpsimd.affine_select(
            out=id128, in_=id128, compare_op=ne, fill=1.0,
            base=0, channel_multiplier=1, pattern=[[-1, 128]],
        )

        # ---- persistent activations ------------------------------------
        QT = bigp.tile([128, T], bf16)   # head0 rows 0-63, head1 rows 64-127
        KT = bigp.tile([128, T], bf16)
        # V natural layout, both heads fused: [j-in-block, head, jb, 65]
        # with the softmax-denominator ones column at slot 64.
        Vboth = bigp.tile([128, HPC, JBN, HS + 1], bf16)
        onesb = constp.tile([128, JBN], bf16)
        nc.gpsimd.memset(onesb, 1.0)
        for h in range(HPC):
            nc.vector.tensor_copy(Vboth[:, h, :, HS], onesb)

        def prefetch_xt(tcj):
            nxt = slice(tcj * 512, (tcj + 1) * 512)
            xtn = xpool.tile([128, DC, 512], bf16, tag="xt", name=f"xt{tcj}")
            nc.sync.dma_start(out=xtn[:, 0:4, :], in_=xTr[:, 0:4, nxt])
            nc.scalar.dma_start(out=xtn[:, 4:8, :], in_=xTr[:, 4:8, nxt])
            xts.append(xtn)

        def emit_qk_part(tcj):
            ts = slice(tcj * 512, (tcj + 1) * 512)
            xt = xts[tcj]
            for fc, dest in ((0, QT), (1, KT)):
                ps = pp.tile([128, 512], f32, tag="p", name=f"ps{fc}_{tcj}")
                for dc in range(DC):
                    nc.tensor.matmul(
                        ps,
                        lhsT=w6sb[:, dc, fc * 128:(fc + 1) * 128],
                        rhs=xt[:, dc, :],
                        start=(dc == 0), stop=(dc == DC - 1),
                    )
                nc.scalar.copy(dest[:, ts], ps)

        def emit_v_mm(tcj):
            xt = xts[tcj]
            psv = pp.tile([128, 512], f32, tag="p", name=f"psv_{tcj}")
            for dc in range(DC):
                nc.tensor.matmul(
                    psv,
                    lhsT=w6sb[:, dc, 256:384],
                    rhs=xt[:, dc, :],
                    start=(dc == 0), stop=(dc == DC - 1),
                )
            vts = vtp.tile([128, 512], bf16, tag="vts", name=f"vts_{tcj}")
            nc.scalar.copy(vts, psv)
            return vts

        def emit_v_transposes(tcj, vts):
            for q in range(4):
                jb = tcj * 4 + q
                ptv = pp.tile([128, 128], bf16, tag="p", name=f"ptv_{tcj}_{q}")
                nc.tensor.transpose(
                    ptv,
                    in_=vts[:, q * 128:(q + 1) * 128],
                    identity=id128,
                )
                # one strided copy drops both heads' V rows into Vboth
                nc.vector.tensor_copy(
                    Vboth[:, :, jb, 0:HS],
                    ptv[:, :].rearrange("p (h s) -> p h s", h=HPC),
                )

        # exp tiles moved from the default odd-jb ACT assignment to DVE to
        # balance engine time (ACT also carries the QT/KT/vts/ot copies).
        _act_to_dve = {(7, 1), (7, 9), (7, 17), (7, 25), (6, 1), (6, 17)}

        def emit_attn_block(ib, slot_work):
            isl = slice(ib * 512, (ib + 1) * 512)
            njb = 4 * (ib + 1)
            po = [
                op.tile([65, 512], f32, tag="o", name=f"po{h}_{ib}")
                for h in range(HPC)
            ]
            pending = []
            for jb in range(njb):
                for wfn in slot_work.get(jb, ()):
                    wfn()
                # Diagonal blocks only need i >= j: trim the i range to
                # [128q, 512) (the columns below were fully masked anyway).
                q = jb - 4 * ib
                off = 128 * q if q > 0 else 0
                tisl = slice(ib * 512 + off, (ib + 1) * 512)
                ps = sp.tile([128, 2, 512], f32, tag="s", name=f"s_{ib}_{jb}")
                for h in range(HPC):
                    nc.tensor.matmul(
                        ps[:, h, off:],
                        lhsT=QT[h * 64:(h + 1) * 64, jb * 128:(jb + 1) * 128],
                        rhs=KT[h * 64:(h + 1) * 64, tisl],
                        start=True, stop=True,
                        tile_position=(h * 64, 0),
                    )
                es = esp.tile([128, 2, 512], bf16, tag="es", name=f"es_{ib}_{jb}")
                # exp split: even jb -> DVE Schraudolph, odd jb -> ACT exact;
                # i-block 0 always exact (few keys -> worst error averaging).
                use_act = ((jb % 2 == 1) or (ib == 0)) \
                    and (ib, jb) not in _act_to_dve
                if ib == IB - 1 and jb >= njb - 2:
                    # kernel tail: halve the final exp drain latency by
                    # splitting the tile across both engines.
                    nc.scalar.activation(es[:, 0:1, off:], ps[:, 0:1, off:],
                                         Exp, scale=LN2_128)
                    nc.vector.tensor_scalar(
                        out=es[:, 1:2, off:].bitcast(i16),
                        in0=ps[:, 1:2, off:],
                        scalar1=BPRIME, scalar2=None, op0=add,
                    )
                elif use_act:
                    nc.scalar.activation(es[:, :, off:], ps[:, :, off:], Exp,
                                         scale=LN2_128)
                else:
                    nc.vector.tensor_scalar(
                        out=es[:, :, off:].bitcast(i16),
                        in0=ps[:, :, off:],
                        scalar1=BPRIME,
                        scalar2=None,
                        op0=add,
                    )
                if q >= 0:  # diagonal block: zero out j > i entries (gpsimd).
                    # Only the first 128 trimmed columns can violate i >= j
                    # (j <= 127); beyond them the predicate is always true.
                    nc.gpsimd.affine_select(
                        out=es[:, :, off:off + 128], in_=es[:, :, off:off + 128],
                        compare_op=ge, fill=0.0,
                        base=0, channel_multiplier=-1,
                        pattern=[[0, 2], [1, 128]],
                    )
                # AV runs 2 j-blocks behind the scores so the in-order PE
                # stream never stalls on the exp latency of the newest tile.
                if len(pending) >= 2:
                    pjb, pes, poff = pending.pop(0)
                    for h in range(HPC):
                        nc.tensor.matmul(
                            po[h][:, poff:],
                            lhsT=Vboth[:, h, pjb, :],
                            rhs=pes[:, h, poff:],
                            start=(pjb == 0), stop=False,
                        )
                pending.append((jb, es, off))
            # next chunk's QK (and any other deferred work) is emitted here,
            # between the last scores and the AV flush, so the PE chews on
            # projection matmuls while the final exps drain.
            for wfn in slot_work.get(njb, ()):
                wfn()
            while pending:
                pjb, pes, poff = pending.pop(0)
                for h in range(HPC):
                    nc.tensor.matmul(
                        po[h][:, poff:],
                        lhsT=Vboth[:, h, pjb, :],
                        rhs=pes[:, h, poff:],
                        start=(pjb == 0), stop=(not pending),
                    )
            # evacuate the unnormalized O^T + denominator row; the host does
            # the (tiny) divide and the un-transpose during the gather.
            # h0 via ACT, h1 via DVE so the drain runs on both engines.
            for h in range(HPC):
                ot = finp.tile([65, 512], f32, tag="ot", name=f"ot{h}_{ib}")
                if h == 0:
                    nc.scalar.copy(ot, po[h])
                    nc.sync.dma_start(out=out[h * 65:(h + 1) * 65, isl], in_=ot)
                else:
                    nc.vector.tensor_copy(ot, po[h])
                    nc.scalar.dma_start(out=out[h * 65:(h + 1) * 65, isl], in_=ot)

        # Staircase: attention block k only depends on projection chunks <= k.
        # QK of chunk k+1 is emitted at the END of block k (after its last
        # scores, before the AV flush) so the PE never idles at block
        # boundaries; the V part of chunk k (first needed by AV jb=4k) is
        # sprinkled into the block body, with its transposes deferred two
        # slots so the PE never waits on the vts evacuation copy.  x chunks
        # are prefetched two blocks ahead so the DGE rings stay ahead of
        # the projections.
        prefetch_xt(1)
        emit_qk_part(0)
        vts_box = {}

        def emit_v_mm_slot(k):
            vts_box[k] = emit_v_mm(k)

        def emit_v_tr_slot(k):
            emit_v_transposes(k, vts_box.pop(k))

        for k in range(TC):
            njb = 4 * (k + 1)
            if k == 0:
                # block 0's first AV (emitted at slot 2) needs Vboth[0]:
                # keep the whole V part up front.
                sw = {1: [lambda k=k: (emit_v_mm_slot(k), emit_v_tr_slot(k))]}
            else:
                sw = {1: [lambda k=k: emit_v_mm_slot(k)],
                      3: [lambda k=k: emit_v_tr_slot(k)]}
            if k + 2 < TC:
                sw.setdefault(0, []).append(lambda k=k: prefetch_xt(k + 2))
            if k + 1 < TC:
                sw.setdefault(njb, []).append(lambda k=k: emit_qk_part(k + 1))
            emit_attn_block(k, sw)


# walrus engine-instruction encodings have a single sync-wait slot; hoist
# extra waits onto per-wait NoOps for everything except generated NoOps.
_NO_HOIST_TYPES = frozenset({"InstNoOp"})


def _pair_ldweights(nc):
    """Fuse the score-pair weight loads into one full-array LDWEIGHTS.

    The pattern LDW(rows 0-63), MM(tile 0), LDW(rows 64-127), MM(tile 64)
    loads two half-array weight tiles whose SBUF sources are contiguous
    (QT keeps head0 on partitions 0-63 and head1 on 64-127).  A single
    128-partition LDWEIGHTS feeds both tile-position matmuls, occupies
    one weight-buffer slot instead of two, and lets the next weight load
    start in the background while the pair streams.
    """
    import bass_rust

    for f in nc.m.functions:
        for blk in f.blocks:
            insts = blk.instructions
            out = []
            changed = False
            i = 0
            while i < len(insts):
                if i + 3 < len(insts):
                    a, b, c, d = insts[i:i + 4]
                    if (
                        type(a).__name__ == "InstLdweights"
                        and type(b).__name__ == "InstMatmult"
                        and type(c).__name__ == "InstLdweights"
                        and type(d).__name__ == "InstMatmult"
                        and b.tile_position is not None
                        and c.tile_position is not None
                        and b.tile_position[0] == 0
                        and c.tile_position[0] == 64
                        and b.tile_size is not None
                        and b.tile_size[0] <= 64
                        and a.tile_position == (0, 0)
                        and c.tile_position == (64, 0)
                        and a.ins[0].memref == c.ins[0].memref
                        and a.ins[0].ap == c.ins[0].ap
                        and c.ins[0].offset
                        == a.ins[0].offset + 64 * list(a.ins[0].ap)[0][0]
                    ):
                        pap = a.ins[0]
                        pattern = [list(p) for p in pap.ap]
                        pattern[0][1] = 128
                        pap.ap = pattern
                        a.tile_size = (128, pattern[1][1])
                        # fold any syncs of the dropped LDW into the kept one
                        csi = c.sync_info
                        if csi is not None and (csi.on_wait or csi.on_update):
                            asi = a.sync_info
                            w = list(csi.on_wait)
                            u = list(csi.on_update)
                            if asi is not None:
                                w = list(asi.on_wait) + w
                                u = list(asi.on_update) + u
                            a.sync_info = bass_rust.SyncInfo(
                                on_wait=w, on_update=u
                            )
                        out.extend([a, b, d])
                        changed = True
                        i += 4
                        continue
                out.append(insts[i])
                i += 1
            if changed:
                blk.instructions = out


def _legalize_waits(nc):
    """Hoist multi-waits off engine instructions onto preceding NoOps.

    Most walrus instruction encodings (S3_LW matmul, DMA, ACT, DVE, drain)
    only have room for a single sync-wait command; Tile freely attaches
    several. Waits execute on the engine's sequencer in program order, so
    moving them to immediately-preceding NoOps is semantics-preserving.
    """
    import bass_rust

    for f in nc.m.functions:
        for blk in f.blocks:
            out = []
            changed = False
            for inst in blk.instructions:
                si = getattr(inst, "sync_info", None)
                if (
                    type(inst).__name__ not in _NO_HOIST_TYPES
                    and si is not None
                    and len(si.on_wait) >= 2
                ):
                    waits = list(si.on_wait)
                    for k, w in enumerate(waits[:-1]):
                        nop = bass_rust.InstNoOp(name=f"{inst.name}_hoistw{k}")
                        nop.engine = inst.engine
                        nop.sync_info = bass_rust.SyncInfo(
                            on_wait=[w], on_update=[]
                        )
                        out.append(nop)
                    si.on_wait = [waits[-1]]
                    changed = True
                out.append(inst)
            if changed:
                blk.instructions = out


def _build_program():
    import concourse.bass as bass
    import concourse.mybir as mybir
    import concourse.tile as tile

    nc = bass.Bass("TRN2", target_bir_lowering=False, debug=False, num_devices=NCORES)
    xT = nc.dram_tensor("xT", [D, T], mybir.dt.bfloat16, kind="ExternalInput").ap()
    w6 = nc.dram_tensor("w6", [D, 6 * HS], mybir.dt.bfloat16, kind="ExternalInput").ap()
    out = nc.dram_tensor("outR", [HPC * (HS + 1), T], mybir.dt.float32, kind="ExternalOutput").ap()

    with tile.TileContext(nc) as tc:
        _emit(tc, nc, xT, w6, out)
    _pair_ldweights(nc)
    _legalize_waits(nc)
    return nc


def _in_maps(x, Wk, Wq, Wv):
    import ml_dtypes

    bf = ml_dtypes.bfloat16
    xTh = np.ascontiguousarray(np.asarray(x, dtype=np.float32).T.astype(bf))
    maps = []
    for c in range(NCORES):
        h0, h1 = HPC * c, HPC * c + 1
        W6 = np.concatenate(
            [Wq[h0] * APRIME, Wq[h1] * APRIME, Wk[h0], Wk[h1], Wv[h0], Wv[h1]],
            axis=1,
        ).astype(bf)
        maps.append({"xT": xTh, "w6": np.ascontiguousarray(W6)})
    return maps


def get_program():
    global _cached_nc
    if _cached_nc is None:
        _cached_nc = _build_program()
    return _cached_nc


def kernel(x, Wk, Wq, Wv):
    import os

    from concourse.bass_utils import run_bass_kernel_spmd

    # The neuronx-cc compile cache keys on tensor shapes only (not BIR
    # content), so a shared cache can serve a stale NEFF for a same-shape
    # program. Force a fresh compile; repeat calls in one process still hit
    # the in-memory jit cache.
    os.environ.setdefault("NEURON_FORCE_RECOMPILE", "1")

    nc = get_program()
    res = run_bass_kernel_spmd(nc, _in_maps(x, Wk, Wq, Wv), core_ids=list(range(NCORES)))
    cols = []
    for c in range(NCORES):
        raw = res.results[c]["outR"]  # [2*65, T]: per head 64 rows O^T + denom
        for h in range(HPC):
            o = raw[h * 65:h * 65 + HS]
            den = raw[h * 65 + HS:h * 65 + HS + 1]
            cols.append((o / den).T)
    return np.ascontiguousarray(np.concatenate(cols, axis=1), dtype=np.float32)


# revision 33
# speedup vs baseline: 1.0086x; 1.0086x over previous
"""Multi-head causal attention (scores = K @ Q^T variant) on 8 TRN2 NeuronCores.

Head-parallel sharding: core c computes heads (2c, 2c+1) end-to-end and the
host concatenates the per-core [T, 128] outputs along the feature axis.

Per-core kernel layout notes:
  - Host passes x transposed ([D, T]) and pre-cast to bf16 so every
    projection matmul has the contraction dim (d) on SBUF partitions with
    zero on-device transposes.
  - Q^T/K^T are stored [128, T] with head0 on partitions 0-63 and head1 on
    64-127, letting the S^T matmuls for both heads run concurrently on
    disjoint PE row-groups (tile_position).
  - Scores are computed transposed (S^T[j, i] = Q_j . K_i) so that the AV
    contraction (over j) lands on the partition axis with no transposes.
  - V is stored in natural layout with a fused ones-column ([V | 1]) so a
    single AV matmul produces both the weighted sum and the softmax
    denominator (PSUM row 64).
  - Wq is pre-scaled on the host by A' = 128*log2(e)/sqrt(HS) so the scores
    PSUM holds 128*log2(exp(s/sqrt(HS))).  The exp is then split across two
    engines to halve the softmax bottleneck:
      * ACT tiles: activation(Exp, scale=ln2/128) - exact.
      * DVE tiles: one tensor_scalar(+B') writing int16 into a bitcast view
        of the bf16 es tile - the int16 bits ARE the bf16 representation of
        2^(ps/128) up to a piecewise-linear mantissa approximation
        (Schraudolph).  Max exp error ~3.5%, which softmax normalization
        mostly cancels; i-block 0 (rows with few keys, worst averaging) is
        pinned to the exact ACT path.  Measured end-to-end rel err ~6e-3.
  - Diagonal-block causal masking runs on the otherwise-idle GPSIMD engine
    (affine_select with fill=0 directly on the es tile).
  - Matmul operands are bf16; PSUM accumulation and normalization are fp32.
  - Projection chunk k and attention i-block k are emitted interleaved
    (i-block k only needs x columns < 512*(k+1)) from one pool set sized to
    exactly 8 PSUM banks, so projections and attention overlap instead of
    serializing on a pool boundary.
"""

import numpy as np

T, D, H, HS = 4096, 1024, 16, 64
NCORES = 8
HPC = H // NCORES  # heads per core = 2
DC = D // 128      # 8 contraction chunks
TC = T // 512      # 8 t-chunks for projections
IB = T // 512      # 8 i-blocks (512 output rows each)
JBN = T // 128     # 32 j-blocks (128 keys each)

# Schraudolph constants: ps = 128*log2(exp(s/8)); es_bits = round(ps + B')
APRIME = 128.0 * np.log2(np.e) / 8.0
CSHIFT = 0.0352                      # minimax relative-error centering
BPRIME = 128.0 * (127.0 - CSHIFT)
LN2_128 = float(np.log(2.0) / 128.0)

_cached_nc = None


def _emit(tc, nc, xT, w6, out):
    import concourse.bass as bass  # noqa: F401
    import concourse.mybir as mybir

    f32 = mybir.dt.float32
    bf16 = mybir.dt.bfloat16
    i16 = mybir.dt.int16
    Exp = mybir.ActivationFunctionType.Exp
    ne = mybir.AluOpType.not_equal
    ge = mybir.AluOpType.is_ge
    add = mybir.AluOpType.add

    with (
        tc.tile_pool(name="const", bufs=1) as constp,
        tc.tile_pool(name="wpool", bufs=1) as wpool,
        tc.tile_pool(name="bigp", bufs=1) as bigp,
        tc.tile_pool(name="xpool", bufs=3) as xpool,
        tc.tile_pool(name="vtp", bufs=2) as vtp,
        tc.tile_pool(name="esp", bufs=8) as esp,
        tc.tile_pool(name="finp", bufs=4) as finp,
        # PSUM budget (8 banks total): s 2x2 + o 2x1 + p 2x1 (shared with
        # the V-transpose tiles so projections get double-buffering).
        tc.tile_pool(name="sp", bufs=2, space="PSUM") as sp,
        tc.tile_pool(name="op", bufs=2, space="PSUM") as op,
        tc.tile_pool(name="pp", bufs=2, space="PSUM") as pp,
    ):
        # ---- input DMAs for weights + first x chunk go first ------------
        w6r = w6.rearrange("(dc p) f -> p dc f", p=128)
        xTr = xT.rearrange("(dc p) t -> p dc t", p=128)
        w6sb = wpool.tile([128, DC, 6 * HS], bf16)
        xts = []
        xt0 = xpool.tile([128, DC, 512], bf16, tag="xt", name="xt0")
        # Per-dc interleave on two DGE queues (x on sync, w6 on scalar) so
        # the dc-k projection matmuls start as soon as pair k has landed,
        # with neither queue blocked behind a bulk transfer.
        for dc in range(DC):
            nc.sync.dma_start(out=xt0[:, dc, :], in_=xTr[:, dc, 0:512])
            nc.scalar.dma_start(out=w6sb[:, dc, :], in_=w6r[:, dc, :])
        xts.append(xt0)

        # ---- constants (gpsimd; overlaps the DMAs) ----------------------
        # id128: full 128x128 identity for the combined V transposes.
        id128 = constp.tile([128, 128], bf16)
        nc.gpsimd.memset(id128, 0.0)
        nc.gpsimd.affine_select(
            out=id128, in_=id128, compare_op=ne, fill=1.0,
            base=0, channel_multiplier=1, pattern=[[-1, 128]],
        )

        # ---- persistent activations ------------------------------------
        QT = bigp.tile([128, T], bf16)   # head0 rows 0-63, head1 rows 64-127
        KT = bigp.tile([128, T], bf16)
        # V natural layout, both heads fused: [j-in-block, head, jb, 65]
        # with the softmax-denominator ones column at slot 64.
        Vboth = bigp.tile([128, HPC, JBN, HS + 1], bf16)
        onesb = constp.tile([128, JBN], bf16)
        nc.gpsimd.memset(onesb, 1.0)
        for h in range(HPC):
            nc.vector.tensor_copy(Vboth[:, h, :, HS], onesb)

        def prefetch_xt(tcj):
            nxt = slice(tcj * 512, (tcj + 1) * 512)
            xtn = xpool.tile([128, DC, 512], bf16, tag="xt", name=f"xt{tcj}")
            nc.sync.dma_start(out=xtn[:, 0:4, :], in_=xTr[:, 0:4, nxt])
            nc.scalar.dma_start(out=xtn[:, 4:8, :], in_=xTr[:, 4:8, nxt])
            xts.append(xtn)

        def emit_qk_part(tcj):
            ts = slice(tcj * 512, (tcj + 1) * 512)
            xt = xts[tcj]
            for fc, dest in ((0, QT), (1, KT)):
                ps = pp.tile([128, 512], f32, tag="p", name=f"ps{fc}_{tcj}")
                for dc in range(DC):
                    nc.tensor.matmul(
                        ps,
                        lhsT=w6sb[:, dc, fc * 128:(fc + 1) * 128],
                        rhs=xt[:, dc, :],
                        start=(dc == 0), stop=(dc == DC - 1),
                    )
                nc.scalar.copy(dest[:, ts], ps)

        def emit_v_part(tcj):
            xt = xts[tcj]
            psv = pp.tile([128, 512], f32, tag="p", name=f"psv_{tcj}")
            for dc in range(DC):
                nc.tensor.matmul(
                    psv,
                    lhsT=w6sb[:, dc, 256:384],
                    rhs=xt[:, dc, :],
                    start=(dc == 0), stop=(dc == DC - 1),
                )
            vts = vtp.tile([128, 512], bf16, tag="vts", name=f"vts_{tcj}")
            nc.scalar.copy(vts, psv)
            for q in range(4):
                jb = tcj * 4 + q
                ptv = pp.tile([128, 128], bf16, tag="p", name=f"ptv_{tcj}_{q}")
                nc.tensor.transpose(
                    ptv,
                    in_=vts[:, q * 128:(q + 1) * 128],
                    identity=id128,
                )
                # one strided copy drops both heads' V rows into Vboth
                nc.vector.tensor_copy(
                    Vboth[:, :, jb, 0:HS],
                    ptv[:, :].rearrange("p (h s) -> p h s", h=HPC),
                )

        # exp tiles moved from the default odd-jb ACT assignment to DVE to
        # balance engine time (ACT also carries the QT/KT/vts/ot copies).
        _act_to_dve = {(7, 1), (7, 9), (7, 17), (7, 25), (6, 1), (6, 17)}

        def emit_attn_block(ib, slot_work):
            isl = slice(ib * 512, (ib + 1) * 512)
            njb = 4 * (ib + 1)
            po = [
                op.tile([65, 512], f32, tag="o", name=f"po{h}_{ib}")
                for h in range(HPC)
            ]
            pending = []
            for jb in range(njb):
                for wfn in slot_work.get(jb, ()):
                    wfn()
                # Diagonal blocks only need i >= j: trim the i range to
                # [128q, 512) (the columns below were fully masked anyway).
                q = jb - 4 * ib
                off = 128 * q if q > 0 else 0
                tisl = slice(ib * 512 + off, (ib + 1) * 512)
                ps = sp.tile([128, 2, 512], f32, tag="s", name=f"s_{ib}_{jb}")
                for h in range(HPC):
                    nc.tensor.matmul(
                        ps[:, h, off:],
                        lhsT=QT[h * 64:(h + 1) * 64, jb * 128:(jb + 1) * 128],
                        rhs=KT[h * 64:(h + 1) * 64, tisl],
                        start=True, stop=True,
                        tile_position=(h * 64, 0),
                    )
                es = esp.tile([128, 2, 512], bf16, tag="es", name=f"es_{ib}_{jb}")
                # exp split: even jb -> DVE Schraudolph, odd jb -> ACT exact;
                # i-block 0 always exact (few keys -> worst error averaging).
                use_act = ((jb % 2 == 1) or (ib == 0)) \
                    and (ib, jb) not in _act_to_dve
                if ib == IB - 1 and jb >= njb - 2:
                    # kernel tail: halve the final exp drain latency by
                    # splitting the tile across both engines.
                    nc.scalar.activation(es[:, 0:1, off:], ps[:, 0:1, off:],
                                         Exp, scale=LN2_128)
                    nc.vector.tensor_scalar(
                        out=es[:, 1:2, off:].bitcast(i16),
                        in0=ps[:, 1:2, off:],
                        scalar1=BPRIME, scalar2=None, op0=add,
                    )
                elif use_act:
                    nc.scalar.activation(es[:, :, off:], ps[:, :, off:], Exp,
                                         scale=LN2_128)
                else:
                    nc.vector.tensor_scalar(
                        out=es[:, :, off:].bitcast(i16),
                        in0=ps[:, :, off:],
                        scalar1=BPRIME,
                        scalar2=None,
                        op0=add,
                    )
                if q >= 0:  # diagonal block: zero out j > i entries (gpsimd).
                    # Only the first 128 trimmed columns can violate i >= j
                    # (j <= 127); beyond them the predicate is always true.
                    nc.gpsimd.affine_select(
                        out=es[:, :, off:off + 128], in_=es[:, :, off:off + 128],
                        compare_op=ge, fill=0.0,
                        base=0, channel_multiplier=-1,
                        pattern=[[0, 2], [1, 128]],
                    )
                # AV runs 2 j-blocks behind the scores so the in-order PE
                # stream never stalls on the exp latency of the newest tile.
                if len(pending) >= 2:
                    pjb, pes, poff = pending.pop(0)
                    for h in range(HPC):
                        nc.tensor.matmul(
                            po[h][:, poff:],
                            lhsT=Vboth[:, h, pjb, :],
                            rhs=pes[:, h, poff:],
                            start=(pjb == 0), stop=False,
                        )
                pending.append((jb, es, off))
            # next chunk's QK (and any other deferred work) is emitted here,
            # between the last scores and the AV flush, so the PE chews on
            # projection matmuls while the final exps drain.
            for wfn in slot_work.get(njb, ()):
                wfn()
            while pending:
                pjb, pes, poff = pending.pop(0)
                for h in range(HPC):
                    nc.tensor.matmul(
                        po[h][:, poff:],
                        lhsT=Vboth[:, h, pjb, :],
                        rhs=pes[:, h, poff:],
                        start=(pjb == 0), stop=(not pending),
                    )
            # evacuate the unnormalized O^T + denominator row; the host does
            # the (tiny) divide and the un-transpose during the gather.
            # h0 via ACT, h1 via DVE so the drain runs on both engines.
            for h in range(HPC):
                ot = finp.tile([65, 512], f32, tag="ot", name=f"ot{h}_{ib}")
                if h == 0:
                    nc.scalar.copy(ot, po[h])
                    nc.sync.dma_start(out=out[h * 65:(h + 1) * 65, isl], in_=ot)
                else:
                    nc.vector.tensor_copy(ot, po[h])
                    nc.scalar.dma_start(out=out[h * 65:(h + 1) * 65, isl], in_=ot)

        # Staircase: attention block k only depends on projection chunks <= k.
        # QK of chunk k+1 is emitted at the END of block k (after its last
        # scores, before the AV flush) so the PE never idles at block
        # boundaries; the V part of chunk k (first needed by AV jb=4k) and
        # the xt prefetch are sprinkled into the block body.
        emit_qk_part(0)
        for k in range(TC):
            njb = 4 * (k + 1)
            sw = {1: [lambda k=k: emit_v_part(k)]}
            if k + 1 < TC:
                sw[0] = [lambda k=k: prefetch_xt(k + 1)]
                sw[njb] = [lambda k=k: emit_qk_part(k + 1)]
            emit_attn_block(k, sw)


# walrus engine-instruction encodings have a single sync-wait slot; hoist
# extra waits onto per-wait NoOps for everything except generated NoOps.
_NO_HOIST_TYPES = frozenset({"InstNoOp"})


def _pair_ldweights(nc):
    """Fuse the score-pair weight loads into one full-array LDWEIGHTS.

    The pattern LDW(rows 0-63), MM(tile 0), LDW(rows 64-127), MM(tile 64)
    loads two half-array weight tiles whose SBUF sources are contiguous
    (QT keeps head0 on partitions 0-63 and head1 on 64-127).  A single
    128-partition LDWEIGHTS feeds both tile-position matmuls, occupies
    one weight-buffer slot instead of two, and lets the next weight load
    start in the background while the pair streams.
    """
    import bass_rust

    for f in nc.m.functions:
        for blk in f.blocks:
            insts = blk.instructions
            out = []
            changed = False
            i = 0
            while i < len(insts):
                if i + 3 < len(insts):
                    a, b, c, d = insts[i:i + 4]
                    if (
                        type(a).__name__ == "InstLdweights"
                        and type(b).__name__ == "InstMatmult"
                        and type(c).__name__ == "InstLdweights"
                        and type(d).__name__ == "InstMatmult"
                        and b.tile_position is not None
                        and c.tile_position is not None
                        and b.tile_position[0] == 0
                        and c.tile_position[0] == 64
                        and b.tile_size is not None
                        and b.tile_size[0] <= 64
                        and a.tile_position == (0, 0)
                        and c.tile_position == (64, 0)
                        and a.ins[0].memref == c.ins[0].memref
                        and a.ins[0].ap == c.ins[0].ap
                        and c.ins[0].offset
                        == a.ins[0].offset + 64 * list(a.ins[0].ap)[0][0]
                    ):
                        pap = a.ins[0]
                        pattern = [list(p) for p in pap.ap]
                        pattern[0][1] = 128
                        pap.ap = pattern
                        a.tile_size = (128, pattern[1][1])
                        # fold any syncs of the dropped LDW into the kept one
                        csi = c.sync_info
                        if csi is not None and (csi.on_wait or csi.on_update):
                            asi = a.sync_info
                            w = list(csi.on_wait)
                            u = list(csi.on_update)
                            if asi is not None:
                                w = list(asi.on_wait) + w
                                u = list(asi.on_update) + u
                            a.sync_info = bass_rust.SyncInfo(
                                on_wait=w, on_update=u
                            )
                        out.extend([a, b, d])
                        changed = True
                        i += 4
                        continue
                out.append(insts[i])
                i += 1
            if changed:
                blk.instructions = out


def _legalize_waits(nc):
    """Hoist multi-waits off engine instructions onto preceding NoOps.

    Most walrus instruction encodings (S3_LW matmul, DMA, ACT, DVE, drain)
    only have room for a single sync-wait command; Tile freely attaches
    several. Waits execute on the engine's sequencer in program order, so
    moving them to immediately-preceding NoOps is semantics-preserving.
    """
    import bass_rust

    for f in nc.m.functions:
        for blk in f.blocks:
            out = []
            changed = False
            for inst in blk.instructions:
                si = getattr(inst, "sync_info", None)
                if (
                    type(inst).__name__ not in _NO_HOIST_TYPES
                    and si is not None
                    and len(si.on_wait) >= 2
                ):
                    waits = list(si.on_wait)
                    for k, w in enumerate(waits[:-1]):
                        nop = bass_rust.InstNoOp(name=f"{inst.name}_hoistw{k}")
                        nop.engine = inst.engine
                        nop.sync_info = bass_rust.SyncInfo(
                            on_wait=[w], on_update=[]
                        )
                        out.append(nop)
                    si.on_wait = [waits[-1]]
                    changed = True
                out.append(inst)
            if changed:
                blk.instructions = out


def _build_program():
    import concourse.bass as bass
    import concourse.mybir as mybir
    import concourse.tile as tile

    nc = bass.Bass("TRN2", target_bir_lowering=False, debug=False, num_devices=NCORES)
    xT = nc.dram_tensor("xT", [D, T], mybir.dt.bfloat16, kind="ExternalInput").ap()
    w6 = nc.dram_tensor("w6", [D, 6 * HS], mybir.dt.bfloat16, kind="ExternalInput").ap()
    out = nc.dram_tensor("outR", [HPC * (HS + 1), T], mybir.dt.float32, kind="ExternalOutput").ap()

    with tile.TileContext(nc) as tc:
        _emit(tc, nc, xT, w6, out)
    _pair_ldweights(nc)
    _legalize_waits(nc)
    return nc


def _in_maps(x, Wk, Wq, Wv):
    import ml_dtypes

    bf = ml_dtypes.bfloat16
    xTh = np.ascontiguousarray(np.asarray(x, dtype=np.float32).T.astype(bf))
    maps = []
    for c in range(NCORES):
        h0, h1 = HPC * c, HPC * c + 1
        W6 = np.concatenate(
            [Wq[h0] * APRIME, Wq[h1] * APRIME, Wk[h0], Wk[h1], Wv[h0], Wv[h1]],
            axis=1,
        ).astype(bf)
        maps.append({"xT": xTh, "w6": np.ascontiguousarray(W6)})
    return maps


def get_program():
    global _cached_nc
    if _cached_nc is None:
        _cached_nc = _build_program()
    return _cached_nc


def kernel(x, Wk, Wq, Wv):
    import os

    from concourse.bass_utils import run_bass_kernel_spmd

    # The neuronx-cc compile cache keys on tensor shapes only (not BIR
    # content), so a shared cache can serve a stale NEFF for a same-shape
    # program. Force a fresh compile; repeat calls in one process still hit
    # the in-memory jit cache.
    os.environ.setdefault("NEURON_FORCE_RECOMPILE", "1")

    nc = get_program()
    res = run_bass_kernel_spmd(nc, _in_maps(x, Wk, Wq, Wv), core_ids=list(range(NCORES)))
    cols = []
    for c in range(NCORES):
        raw = res.results[c]["outR"]  # [2*65, T]: per head 64 rows O^T + denom
        for h in range(HPC):
            o = raw[h * 65:h * 65 + HS]
            den = raw[h * 65 + HS:h * 65 + HS + 1]
            cols.append((o / den).T)
    return np.ascontiguousarray(np.concatenate(cols, axis=1), dtype=np.float32)
